# revision 2
# baseline (speedup 1.0000x reference)
"""Trainium2 Bass kernel for nn_CrossAttention_Single (channel-wise cross attention).

Reference math (B=32, N=256, C=4096, H=16, hd=256, SCALE=1/16):
    q = (x1 @ Wq.T) * SCALE ; k = x2 @ Wk.T ; v = x2 @ Wv.T        [B,N,C]
    per (b,h):  S = q_bh^T @ k_bh            [hd,hd]  (contract tokens)
                attn = softmax(S, axis=-1)
                O = attn @ v_bh              [hd(=token row), hd]
    x = O @ Wp.T + bp                        [B,N,C]
    returns (x, attn)

Sharding: data-parallel over batch, 4 batches per core on 8 cores.

Device layout notes:
  - All matmul operands are fed in [contraction, out] SBUF layout; host
    pre-transposes x and W so every DMA is contiguous-row.
  - fp32r everywhere on the PE (full rate at free dim >= 256); the BIR
    verifier's "rounded to FP32r" rule is satisfied by DMA-ing into tiles
    declared float32r and by emitting float32r from on-chip producers.
  - softmax: exp on ACT with accum_out row sums; normalize via DVE
    tensor_scalar; attn^T via PE transpose so O is produced pre-transposed
    ([C, tokens]) for the output projection.
"""

import numpy as np

import concourse.bass as bass
import concourse.mybir as mybir
import concourse.tile as tile
from concourse import bacc
from concourse.bass_utils import run_bass_kernel_spmd

F32 = mybir.dt.float32
F32R = mybir.dt.float32r
AF = mybir.ActivationFunctionType

B, N, C = 32, 256, 4096
H = 16
HD = C // H  # 256
SCALE = HD**-0.5  # 1/16
N_CORES = 8
B_LOC = B // N_CORES  # 4
TOK = B_LOC * N  # 1024 tokens per core
KC = C // 128  # 32 contraction chunks
P = 128

TRACE = False  # test harness flips this for NTFF profiling
_CACHE = {}


def _emit_projection(nc, tc, pools, xt_tiles, w_dram, out_dram, n_cc=8, cw=512):
    """out[tok, cout] = x^T-tiles.T @ w  (fp32r), kc-outer 8-bank PSUM scheme."""
    wpool, opool, pspool = pools
    n_tc = TOK // P  # 8
    for cc in range(n_cc):
        psums = [pspool.tile([P, cw], F32, tag="mm", name=f"mmps{i}") for i in range(n_tc)]
        for kc in range(KC):
            wt = wpool.tile([P, cw], F32R, tag="wt")
            nc.sync.dma_start(
                out=wt[:],
                in_=w_dram[kc * P : (kc + 1) * P, cc * cw : (cc + 1) * cw].bitcast(F32R),
            )
            for t in range(n_tc):
                nc.tensor.matmul(
                    psums[t][:],
                    xt_tiles[kc][:, t * P : (t + 1) * P],
                    wt[:],
                    start=(kc == 0),
                    stop=(kc == KC - 1),
                )
        for t in range(n_tc):
            ot = opool.tile([P, cw], F32, tag="mo")
            nc.vector.tensor_copy(ot[:], psums[t][:])
            nc.gpsimd.dma_start(
                out=out_dram[t * P : (t + 1) * P, cc * cw : (cc + 1) * cw], in_=ot[:]
            )


def build_nc():
    nc = bacc.Bacc("TRN2", target_bir_lowering=False, debug=False)

    x1T = nc.dram_tensor("x1T", [C, TOK], F32, kind="ExternalInput").ap()
    x2T = nc.dram_tensor("x2T", [C, TOK], F32, kind="ExternalInput").ap()
    wqT = nc.dram_tensor("wqT", [C, C], F32, kind="ExternalInput").ap()
    wkT = nc.dram_tensor("wkT", [C, C], F32, kind="ExternalInput").ap()
    wvT = nc.dram_tensor("wvT", [C, C], F32, kind="ExternalInput").ap()
    wpT = nc.dram_tensor("wpT", [C, C], F32, kind="ExternalInput").ap()
    bpr = nc.dram_tensor("bpr", [1, C], F32, kind="ExternalInput").ap()
    onesr = nc.dram_tensor("onesr", [1, P], F32, kind="ExternalInput").ap()
    ident = nc.dram_tensor("ident", [P, P], F32, kind="ExternalInput").ap()

    attn_out = nc.dram_tensor(
        "attn_out", [B_LOC * H, HD, HD], F32, kind="ExternalOutput"
    ).ap()
    x_out = nc.dram_tensor("x_out", [TOK, C], F32, kind="ExternalOutput").ap()

    with tile.TileContext(nc) as tc:
        with tc.tile_pool(name="dram", bufs=1, space="DRAM") as dram:
            Qd = dram.tile([TOK, C], F32)
            Kd = dram.tile([TOK, C], F32)
            Vd = dram.tile([TOK, C], F32)
            OTd = dram.tile([C, TOK], F32)

            with tc.tile_pool(name="const", bufs=1) as cpool:
                id_t = cpool.tile([P, P], F32R, tag="id")
                ones_t = cpool.tile([1, P], F32R, tag="ones")
                bp_t = cpool.tile([1, C], F32R, tag="bp")
                nc.sync.dma_start(out=id_t[:], in_=ident[:].bitcast(F32R))
                nc.sync.dma_start(out=ones_t[:], in_=onesr[:].bitcast(F32R))
                nc.sync.dma_start(out=bp_t[:], in_=bpr[:].bitcast(F32R))

                # ---------------- Phase A + B1: Q, K, V projections ---------
                with (
                    tc.tile_pool(name="xres", bufs=KC) as xres,
                    tc.tile_pool(name="wstream", bufs=4) as wpool,
                    tc.tile_pool(name="mmout", bufs=4) as opool,
                    tc.tile_pool(name="psA", bufs=8, space="PSUM") as pspool,
                ):
                    pools = (wpool, opool, pspool)
                    x1_tiles = []
                    for kc in range(KC):
                        xt = xres.tile([P, TOK], F32R, tag="xt")
                        nc.sync.dma_start(
                            out=xt[:], in_=x1T[kc * P : (kc + 1) * P, :].bitcast(F32R)
                        )
                        x1_tiles.append(xt)
                    _emit_projection(nc, tc, pools, x1_tiles, wqT, Qd)

                    x2_tiles = []
                    for kc in range(KC):
                        xt = xres.tile([P, TOK], F32R, tag="xt")
                        nc.sync.dma_start(
                            out=xt[:], in_=x2T[kc * P : (kc + 1) * P, :].bitcast(F32R)
                        )
                        x2_tiles.append(xt)
                    _emit_projection(nc, tc, pools, x2_tiles, wkT, Kd)
                    _emit_projection(nc, tc, pools, x2_tiles, wvT, Vd)

                # ---------------- Phase B2: attention per (b, h) ------------
                with (
                    tc.tile_pool(name="qkv", bufs=12) as qkv,
                    tc.tile_pool(name="soft", bufs=8) as soft,
                    tc.tile_pool(name="att", bufs=8) as att,
                    tc.tile_pool(name="psS", bufs=4, space="PSUM") as psS,
                    tc.tile_pool(name="psT", bufs=2, space="PSUM") as psT,
                    tc.tile_pool(name="psO", bufs=2, space="PSUM") as psO,
                ):
                    for b in range(B_LOC):
                        for h in range(H):
                            r0 = b * N
                            c0 = h * HD
                            q = [qkv.tile([P, HD], F32R, tag="q", name=f"q{i}") for i in range(2)]
                            k = [qkv.tile([P, HD], F32R, tag="k", name=f"k{i}") for i in range(2)]
                            v = [qkv.tile([P, HD], F32R, tag="v", name=f"v{i}") for i in range(2)]
                            for nc_i in range(2):
                                rows = slice(r0 + nc_i * P, r0 + (nc_i + 1) * P)
                                cols = slice(c0, c0 + HD)
                                nc.sync.dma_start(
                                    out=q[nc_i][:], in_=Qd[rows, cols].bitcast(F32R)
                                )
                                nc.sync.dma_start(
                                    out=k[nc_i][:], in_=Kd[rows, cols].bitcast(F32R)
                                )
                                nc.sync.dma_start(
                                    out=v[nc_i][:], in_=Vd[rows, cols].bitcast(F32R)
                                )
                            # S[i,j] chunks + softmax
                            an = []  # normalized attn tiles (f32r), i-chunked
                            for ic in range(2):
                                s_ps = psS.tile([P, HD], F32, tag="s")
                                for t in range(2):
                                    nc.tensor.matmul(
                                        s_ps[:],
                                        q[t][:, ic * P : (ic + 1) * P],
                                        k[t][:],
                                        start=(t == 0),
                                        stop=(t == 1),
                                    )
                                e_t = soft.tile([P, HD], F32, tag="e")
                                ssum = soft.tile([P, 1], F32, tag="ss")
                                nc.scalar.activation(
                                    e_t[:], s_ps[:], AF.Exp, scale=SCALE,
                                    accum_out=ssum[:],
                                )
                                rec = soft.tile([P, 1], F32, tag="rc")
                                nc.vector.reciprocal(rec[:], ssum[:])
                                a_t = att.tile([P, HD], F32R, tag="an")
                                nc.vector.tensor_scalar_mul(a_t[:], e_t[:], rec[:])
                                nc.gpsimd.dma_start(
                                    out=attn_out[
                                        b * H + h, ic * P : (ic + 1) * P, :
                                    ].bitcast(F32R),
                                    in_=a_t[:],
                                )
                                an.append(a_t)
                            # attn^T via PE transpose: at[jc][:, ic*P:...] tiles
                            at = []
                            for jc in range(2):
                                t_ps = psT.tile([P, HD], F32R, tag="t")
                                for ic in range(2):
                                    nc.tensor.transpose(
                                        t_ps[:, ic * P : (ic + 1) * P],
                                        an[ic][:, jc * P : (jc + 1) * P],
                                        id_t[:],
                                    )
                                t_sb = att.tile([P, HD], F32R, tag="at")
                                nc.scalar.copy(t_sb[:], t_ps[:])
                                at.append(t_sb)
                            # O^T[c, i] = sum_j v[j, c] * attnT[j, i]
                            for ch in range(2):
                                o_ps = psO.tile([P, HD], F32, tag="o")
                                for jc in range(2):
                                    nc.tensor.matmul(
                                        o_ps[:],
                                        v[jc][:, ch * P : (ch + 1) * P],
                                        at[jc][:],
                                        start=(jc == 0),
                                        stop=(jc == 1),
                                    )
                                o_sb = att.tile([P, HD], F32, tag="ot")
                                nc.vector.tensor_copy(o_sb[:], o_ps[:])
                                nc.gpsimd.dma_start(
                                    out=OTd[
                                        c0 + ch * P : c0 + (ch + 1) * P,
                                        r0 : r0 + N,
                                    ],
                                    in_=o_sb[:],
                                )

                # ---------------- Phase C: x = O @ Wp.T + bp ----------------
                with (
                    tc.tile_pool(name="otres", bufs=KC) as otres,
                    tc.tile_pool(name="wstream2", bufs=4) as wpool2,
                    tc.tile_pool(name="xstage", bufs=4) as xstage,
                    tc.tile_pool(name="psC", bufs=8, space="PSUM") as psC,
                ):
                    ot_tiles = []
                    for kc in range(KC):
                        ott = otres.tile([P, TOK], F32R, tag="ott")
                        nc.sync.dma_start(
                            out=ott[:], in_=OTd[kc * P : (kc + 1) * P, :].bitcast(F32R)
                        )
                        ot_tiles.append(ott)
                    cw = 512
                    n_tc = TOK // P
                    for cc in range(8):
                        psums = [psC.tile([P, cw], F32, tag="mm", name=f"cps{i}") for i in range(n_tc)]
                        for t in range(n_tc):
                            nc.tensor.matmul(
                                psums[t][:],
                                ones_t[:],
                                bp_t[:, cc * cw : (cc + 1) * cw],
                                start=True,
                                stop=False,
                            )
                        for kc in range(KC):
                            wt = wpool2.tile([P, cw], F32R, tag="wt2")
                            nc.sync.dma_start(
                                out=wt[:],
                                in_=wpT[
                                    kc * P : (kc + 1) * P, cc * cw : (cc + 1) * cw
                                ].bitcast(F32R),
                            )
                            for t in range(n_tc):
                                nc.tensor.matmul(
                                    psums[t][:],
                                    ot_tiles[kc][:, t * P : (t + 1) * P],
                                    wt[:],
                                    start=False,
                                    stop=(kc == KC - 1),
                                )
                        for t in range(n_tc):
                            xt_o = xstage.tile([P, cw], F32, tag="xo")
                            nc.vector.tensor_copy(xt_o[:], psums[t][:])
                            nc.gpsimd.dma_start(
                                out=x_out[
                                    t * P : (t + 1) * P, cc * cw : (cc + 1) * cw
                                ],
                                in_=xt_o[:],
                            )

    nc.compile()
    return nc


def kernel(x1, x2, Wq, Wk, Wv, Wp, bp):
    x1 = np.ascontiguousarray(np.asarray(x1, dtype=np.float32))
    x2 = np.ascontiguousarray(np.asarray(x2, dtype=np.float32))

    if "nc" not in _CACHE:
        _CACHE["nc"] = build_nc()
    nc = _CACHE["nc"]

    wqT = np.ascontiguousarray(np.asarray(Wq, dtype=np.float32).T)
    wkT = np.ascontiguousarray(np.asarray(Wk, dtype=np.float32).T)
    wvT = np.ascontiguousarray(np.asarray(Wv, dtype=np.float32).T)
    wpT = np.ascontiguousarray(np.asarray(Wp, dtype=np.float32).T)
    bpr = np.asarray(bp, dtype=np.float32).reshape(1, C)
    onesr = np.ones((1, P), dtype=np.float32)
    ident = np.eye(P, dtype=np.float32)

    in_maps = []
    for c in range(N_CORES):
        bs = slice(c * B_LOC, (c + 1) * B_LOC)
        x1T = np.ascontiguousarray(x1[bs].reshape(TOK, C).T)
        x2T = np.ascontiguousarray(x2[bs].reshape(TOK, C).T)
        in_maps.append(
            {
                "x1T": x1T,
                "x2T": x2T,
                "wqT": wqT,
                "wkT": wkT,
                "wvT": wvT,
                "wpT": wpT,
                "bpr": bpr,
                "onesr": onesr,
                "ident": ident,
            }
        )

    res = run_bass_kernel_spmd(
        nc, in_maps, list(range(N_CORES)), trace=TRACE
    )
    _CACHE["last_result"] = res

    x_full = np.empty((B, N, C), dtype=np.float32)
    attn_full = np.empty((B, H, HD, HD), dtype=np.float32)
    for c in range(N_CORES):
        r = res.results[c]
        x_full[c * B_LOC : (c + 1) * B_LOC] = r["x_out"].reshape(B_LOC, N, C)
        attn_full[c * B_LOC : (c + 1) * B_LOC] = r["attn_out"].reshape(
            B_LOC, H, HD, HD
        )
    return x_full, attn_full


# revision 4
# speedup vs baseline: 1.1219x; 1.1219x over previous
"""Trainium2 Bass kernel for nn_CrossAttention_Single (channel-wise cross attention).

Reference math (B=32, N=256, C=4096, H=16, hd=256, SCALE=1/16):
    q = (x1 @ Wq.T) * SCALE ; k = x2 @ Wk.T ; v = x2 @ Wv.T        [B,N,C]
    per (b,h):  S = q_bh^T @ k_bh            [hd,hd]  (contract tokens)
                attn = softmax(S, axis=-1)
                O = attn @ v_bh              [hd(=token row), hd]
    x = O @ Wp.T + bp                        [B,N,C]
    returns (x, attn)

Sharding: data-parallel over batch, 4 batches per core on 8 cores.

Device layout notes:
  - All matmul operands are fed in [contraction, out] SBUF layout; host
    pre-transposes x and W so every DMA is contiguous-row.
  - fp32r everywhere on the PE (full rate at free dim >= 256); the BIR
    verifier's "rounded to FP32r" rule is satisfied by DMA-ing into tiles
    declared float32r and by emitting float32r from on-chip producers.
  - softmax: exp on ACT with accum_out row sums; normalize via DVE
    tensor_scalar; attn^T via PE transpose so O is produced pre-transposed
    ([C, tokens]) for the output projection.
  - O^T stays resident in SBUF between the attention phase and the output
    projection (no DRAM round-trip); attention Q/K/V loads are batched per
    head-pair ([128, 512] tiles -> 2KB DMA rows).
"""

import numpy as np

import concourse.bass as bass
import concourse.mybir as mybir
import concourse.tile as tile
from concourse import bacc
from concourse.bass_utils import run_bass_kernel_spmd

F32 = mybir.dt.float32
F32R = mybir.dt.float32r
AF = mybir.ActivationFunctionType

B, N, C = 32, 256, 4096
H = 16
HD = C // H  # 256
SCALE = HD**-0.5  # 1/16
N_CORES = 8
B_LOC = B // N_CORES  # 4
TOK = B_LOC * N  # 1024 tokens per core
KC = C // 128  # 32 contraction chunks
P = 128

TRACE = False  # test harness flips this for NTFF profiling
_CACHE = {}


def _emit_projection_cc(nc, tc, pools, xt_tiles, w_dram, out_dram, cc, cw=512):
    """One cout chunk of out[tok, cout] = x^T-tiles.T @ w (fp32r, 8-bank PSUM).

    out_dram is either a full [TOK, C] AP (sliced at cc*cw) or a per-cc
    [TOK, cw] tile (used as-is)."""
    wpool, opool, pspool = pools
    n_tc = TOK // P  # 8
    per_cc = out_dram.shape[-1] == cw
    psums = [
        pspool.tile([P, cw], F32, tag="mm", name=f"mmps{i}") for i in range(n_tc)
    ]
    for kc in range(KC):
        wt = wpool.tile([P, cw], F32R, tag="wt")
        nc.sync.dma_start(
            out=wt[:],
            in_=w_dram[kc * P : (kc + 1) * P, cc * cw : (cc + 1) * cw].bitcast(F32R),
        )
        for t in range(n_tc):
            nc.tensor.matmul(
                psums[t][:],
                xt_tiles[kc][:, t * P : (t + 1) * P],
                wt[:],
                start=(kc == 0),
                stop=(kc == KC - 1),
            )
    for t in range(n_tc):
        ot = opool.tile([P, cw], F32, tag="mo")
        nc.vector.tensor_copy(ot[:], psums[t][:])
        if per_cc:
            nc.gpsimd.dma_start(out=out_dram[t * P : (t + 1) * P, :], in_=ot[:])
        else:
            nc.gpsimd.dma_start(
                out=out_dram[t * P : (t + 1) * P, cc * cw : (cc + 1) * cw], in_=ot[:]
            )


def build_nc():
    nc = bacc.Bacc("TRN2", target_bir_lowering=False, debug=False)

    x1T = nc.dram_tensor("x1T", [C, TOK], F32, kind="ExternalInput").ap()
    x2T = nc.dram_tensor("x2T", [C, TOK], F32, kind="ExternalInput").ap()
    wqT = nc.dram_tensor("wqT", [C, C], F32, kind="ExternalInput").ap()
    wkT = nc.dram_tensor("wkT", [C, C], F32, kind="ExternalInput").ap()
    wvT = nc.dram_tensor("wvT", [C, C], F32, kind="ExternalInput").ap()
    wpT = nc.dram_tensor("wpT", [C, C], F32, kind="ExternalInput").ap()
    bpr = nc.dram_tensor("bpr", [1, C], F32, kind="ExternalInput").ap()
    onesr = nc.dram_tensor("onesr", [1, P], F32, kind="ExternalInput").ap()
    ident = nc.dram_tensor("ident", [P, P], F32, kind="ExternalInput").ap()

    attn_out = nc.dram_tensor(
        "attn_out", [B_LOC * H, HD, HD], F32, kind="ExternalOutput"
    ).ap()
    x_out = nc.dram_tensor("x_out", [TOK, C], F32, kind="ExternalOutput").ap()

    with tile.TileContext(nc) as tc:
        with tc.tile_pool(name="dram", bufs=1, space="DRAM") as dram:
            Qd = dram.tile([TOK, C], F32)
            Kd = [dram.tile([TOK, 512], F32, name=f"Kd{i}") for i in range(8)]
            Vd = [dram.tile([TOK, 512], F32, name=f"Vd{i}") for i in range(8)]

            with tc.tile_pool(name="const", bufs=1) as cpool:
                id_t = cpool.tile([P, P], F32R, tag="id")
                ones_t = cpool.tile([1, P], F32R, tag="ones")
                bp_t = cpool.tile([1, C], F32R, tag="bp")
                nc.sync.dma_start(out=id_t[:], in_=ident[:].bitcast(F32R))
                nc.sync.dma_start(out=ones_t[:], in_=onesr[:].bitcast(F32R))
                nc.sync.dma_start(out=bp_t[:], in_=bpr[:].bitcast(F32R))

                # ---------------- Phase A + B1: Q, K, V projections ---------
                with (
                    tc.tile_pool(name="xres", bufs=KC + 4) as xres,
                    tc.tile_pool(name="wstream", bufs=6) as wpool,
                    tc.tile_pool(name="mmout", bufs=6) as opool,
                    tc.tile_pool(name="psA", bufs=8, space="PSUM") as pspool,
                ):
                    pools = (wpool, opool, pspool)
                    x1_tiles = []
                    for kc in range(KC):
                        xt = xres.tile([P, TOK], F32R, tag="xt")
                        nc.sync.dma_start(
                            out=xt[:], in_=x1T[kc * P : (kc + 1) * P, :].bitcast(F32R)
                        )
                        x1_tiles.append(xt)
                    for cc in range(8):
                        _emit_projection_cc(nc, tc, pools, x1_tiles, wqT, Qd, cc)

                    x2_tiles = []
                    for kc in range(KC):
                        xt = xres.tile([P, TOK], F32R, tag="xt")
                        nc.sync.dma_start(
                            out=xt[:], in_=x2T[kc * P : (kc + 1) * P, :].bitcast(F32R)
                        )
                        x2_tiles.append(xt)
                    for cc in range(8):
                        _emit_projection_cc(nc, tc, pools, x2_tiles, wkT, Kd[cc], cc)
                        _emit_projection_cc(nc, tc, pools, x2_tiles, wvT, Vd[cc], cc)

                # ------- Phase B2 + C share the resident O^T pool -----------
                with tc.tile_pool(name="otres", bufs=KC) as otres:
                    ot_res = [
                        otres.tile([P, TOK], F32R, tag="ott", name=f"otr{kc}")
                        for kc in range(KC)
                    ]

                    # ------------ Phase B2: attention per (b, head) ---------
                    with (
                        tc.tile_pool(name="qkv", bufs=6) as qkv,
                        tc.tile_pool(name="soft", bufs=6) as soft,
                        tc.tile_pool(name="att", bufs=6) as att,
                        tc.tile_pool(name="psS", bufs=4, space="PSUM") as psS,
                        tc.tile_pool(name="psT", bufs=2, space="PSUM") as psT,
                        tc.tile_pool(name="psO", bufs=2, space="PSUM") as psO,
                    ):
                        for hp in range(H // 2):
                            cp0 = hp * 2 * HD  # head-pair column offset
                            for b in range(B_LOC):
                                r0 = b * N
                                qp = [
                                    qkv.tile([P, 2 * HD], F32R, tag="q", name=f"q{i}")
                                    for i in range(2)
                                ]
                                kp = [
                                    qkv.tile([P, 2 * HD], F32R, tag="k", name=f"k{i}")
                                    for i in range(2)
                                ]
                                vp = [
                                    qkv.tile([P, 2 * HD], F32R, tag="v", name=f"v{i}")
                                    for i in range(2)
                                ]
                                for t in range(2):
                                    rows = slice(r0 + t * P, r0 + (t + 1) * P)
                                    cols = slice(cp0, cp0 + 2 * HD)
                                    nc.sync.dma_start(
                                        out=qp[t][:], in_=Qd[rows, cols].bitcast(F32R)
                                    )
                                    nc.sync.dma_start(
                                        out=kp[t][:],
                                        in_=Kd[hp][rows, :].bitcast(F32R),
                                    )
                                    nc.sync.dma_start(
                                        out=vp[t][:],
                                        in_=Vd[hp][rows, :].bitcast(F32R),
                                    )
                                for hi in range(2):
                                    h = hp * 2 + hi
                                    co = hi * HD  # column offset inside pair tiles
                                    # S[i,j] chunks + softmax
                                    an = []
                                    for ic in range(2):
                                        s_ps = psS.tile([P, HD], F32, tag="s")
                                        for t in range(2):
                                            nc.tensor.matmul(
                                                s_ps[:],
                                                qp[t][:, co + ic * P : co + (ic + 1) * P],
                                                kp[t][:, co : co + HD],
                                                start=(t == 0),
                                                stop=(t == 1),
                                            )
                                        e_t = soft.tile([P, HD], F32, tag="e")
                                        ssum = soft.tile([P, 1], F32, tag="ss")
                                        nc.scalar.activation(
                                            e_t[:],
                                            s_ps[:],
                                            AF.Exp,
                                            scale=SCALE,
                                            accum_out=ssum[:],
                                        )
                                        rec = soft.tile([P, 1], F32, tag="rc")
                                        nc.vector.reciprocal(rec[:], ssum[:])
                                        a_t = att.tile([P, HD], F32R, tag="an")
                                        nc.vector.tensor_scalar_mul(
                                            a_t[:], e_t[:], rec[:]
                                        )
                                        nc.gpsimd.dma_start(
                                            out=attn_out[
                                                b * H + h, ic * P : (ic + 1) * P, :
                                            ].bitcast(F32R),
                                            in_=a_t[:],
                                        )
                                        an.append(a_t)
                                    # attn^T via PE transpose
                                    at = []
                                    for jc in range(2):
                                        t_ps = psT.tile([P, HD], F32R, tag="t")
                                        for ic in range(2):
                                            nc.tensor.transpose(
                                                t_ps[:, ic * P : (ic + 1) * P],
                                                an[ic][:, jc * P : (jc + 1) * P],
                                                id_t[:],
                                            )
                                        t_sb = att.tile([P, HD], F32R, tag="at")
                                        nc.scalar.copy(t_sb[:], t_ps[:])
                                        at.append(t_sb)
                                    # O^T[c, i] = sum_j v[j, c] * attnT[j, i]
                                    # written straight into the resident pool
                                    for ch in range(2):
                                        o_ps = psO.tile([P, HD], F32, tag="o")
                                        for jc in range(2):
                                            nc.tensor.matmul(
                                                o_ps[:],
                                                vp[jc][:, co + ch * P : co + (ch + 1) * P],
                                                at[jc][:],
                                                start=(jc == 0),
                                                stop=(jc == 1),
                                            )
                                        nc.vector.tensor_copy(
                                            ot_res[h * 2 + ch][:, r0 : r0 + N],
                                            o_ps[:],
                                        )

                    # ------------ Phase C: x = O @ Wp.T + bp ----------------
                    with (
                        tc.tile_pool(name="wstream2", bufs=6) as wpool2,
                        tc.tile_pool(name="xstage", bufs=6) as xstage,
                        tc.tile_pool(name="psC", bufs=8, space="PSUM") as psC,
                    ):
                        cw = 512
                        n_tc = TOK // P
                        for cc in range(8):
                            psums = [
                                psC.tile([P, cw], F32, tag="mm", name=f"cps{i}")
                                for i in range(n_tc)
                            ]
                            for t in range(n_tc):
                                nc.tensor.matmul(
                                    psums[t][:],
                                    ones_t[:],
                                    bp_t[:, cc * cw : (cc + 1) * cw],
                                    start=True,
                                    stop=False,
                                )
                            for kc in range(KC):
                                wt = wpool2.tile([P, cw], F32R, tag="wt2")
                                nc.sync.dma_start(
                                    out=wt[:],
                                    in_=wpT[
                                        kc * P : (kc + 1) * P, cc * cw : (cc + 1) * cw
                                    ].bitcast(F32R),
                                )
                                for t in range(n_tc):
                                    nc.tensor.matmul(
                                        psums[t][:],
                                        ot_res[kc][:, t * P : (t + 1) * P],
                                        wt[:],
                                        start=False,
                                        stop=(kc == KC - 1),
                                    )
                            for t in range(n_tc):
                                xt_o = xstage.tile([P, cw], F32, tag="xo")
                                nc.vector.tensor_copy(xt_o[:], psums[t][:])
                                nc.gpsimd.dma_start(
                                    out=x_out[
                                        t * P : (t + 1) * P, cc * cw : (cc + 1) * cw
                                    ],
                                    in_=xt_o[:],
                                )

    nc.compile()
    return nc


def kernel(x1, x2, Wq, Wk, Wv, Wp, bp):
    x1 = np.ascontiguousarray(np.asarray(x1, dtype=np.float32))
    x2 = np.ascontiguousarray(np.asarray(x2, dtype=np.float32))

    if "nc" not in _CACHE:
        _CACHE["nc"] = build_nc()
    nc = _CACHE["nc"]

    wqT = np.ascontiguousarray(np.asarray(Wq, dtype=np.float32).T)
    wkT = np.ascontiguousarray(np.asarray(Wk, dtype=np.float32).T)
    wvT = np.ascontiguousarray(np.asarray(Wv, dtype=np.float32).T)
    wpT = np.ascontiguousarray(np.asarray(Wp, dtype=np.float32).T)
    bpr = np.asarray(bp, dtype=np.float32).reshape(1, C)
    onesr = np.ones((1, P), dtype=np.float32)
    ident = np.eye(P, dtype=np.float32)

    in_maps = []
    for c in range(N_CORES):
        bs = slice(c * B_LOC, (c + 1) * B_LOC)
        x1T = np.ascontiguousarray(x1[bs].reshape(TOK, C).T)
        x2T = np.ascontiguousarray(x2[bs].reshape(TOK, C).T)
        in_maps.append(
            {
                "x1T": x1T,
                "x2T": x2T,
                "wqT": wqT,
                "wkT": wkT,
                "wvT": wvT,
                "wpT": wpT,
                "bpr": bpr,
                "onesr": onesr,
                "ident": ident,
            }
        )

    res = run_bass_kernel_spmd(nc, in_maps, list(range(N_CORES)), trace=TRACE)
    _CACHE["last_result"] = res

    x_full = np.empty((B, N, C), dtype=np.float32)
    attn_full = np.empty((B, H, HD, HD), dtype=np.float32)
    for c in range(N_CORES):
        r = res.results[c]
        x_full[c * B_LOC : (c + 1) * B_LOC] = r["x_out"].reshape(B_LOC, N, C)
        attn_full[c * B_LOC : (c + 1) * B_LOC] = r["attn_out"].reshape(
            B_LOC, H, HD, HD
        )
    return x_full, attn_full
